# revision 12
# baseline (speedup 1.0000x reference)
"""Complex attention (split re/im softmax) on 8 trn2 NeuronCores.

Math per (b,h) pair (L=S=1024, E=D=64):
  scores_re[l,s] = sum_e qr[l,e]kr[s,e] + qi[l,e]ki[s,e]   (x 1/sqrt(E))
  scores_im[l,s] = sum_e qi[l,e]kr[s,e] - qr[l,e]ki[s,e]   (x 1/sqrt(E))
  Ar = softmax_s(scores_re); Ai = softmax_s(scores_im)
  Vre = Ar@vr - Ai@vi ; Vim = Ar@vi + Ai@vr

Kernel strategy (per core: 4 of the 32 (b,h) pairs):
  - Pack the re/im contraction into K=128 matmuls:
      qcat = [qr; qi]^T * scale   [128, L]
      kre  = [kr; ki]^T           [128, S]
      kim  = [-ki; kr]^T          [128, S]
    scoresT (s on partitions, l free) = kre_chunk.T @ qcat, kim_chunk.T @ qcat
  - The P^T = exp(scoresT) stream is the bottleneck: exp only runs on the
    Scalar engine at 1 col/cycle.  Scores psum is organized as a rotating
    pair of [128,1536] tiles (6 banks) so the Tensor engine runs ~2 units
    ahead of ScalarE, and exp units are as wide as psum allows (fewer
    fixed per-instruction overheads).
  - AV for pair p-1 is interleaved into pair p's score stream:
      acc[l,0:129]   += Pre^T_chunk.T @ [vr  | vi | 1]
      acc[l,129:258] += Pim^T_chunk.T @ [-vi | vr | 1]
    (both halves live in one psum bank; col 128/257 are the softmax
    denominators Zr/Zi).  The sign flip in vaug2 makes the final combine a
    single fused op per half:
      V = acc_re[:,0:128]/Zr + acc_im[:,0:128]/Zi = [Vre | Vim]
  - Normalize on VectorE: 2 reciprocals + tensor_scalar mul + one
    scalar_tensor_tensor, then DMA [128,128] f32 per l-chunk.
"""

import numpy as np
import ml_dtypes

import concourse.bass as bass
from concourse import mybir
from concourse.tile import TileContext
from concourse.bass_utils import run_bass_kernel_spmd

B, L, H, E = 4, 1024, 8, 64
S, D = 1024, 64
NCORES = 8
PAIRS = B * H              # 32 (b,h) pairs
PPC = PAIRS // NCORES      # 4 pairs per core
NT = S // 128              # 8 s-tiles
NL = L // 128              # 8 l-chunks
NTP = 2 * NT               # 16 tile-parts (s-tile x re/im) per pair
VW = 132                   # padded vaug width (v0 64 | v1 64 | ones 1 | pad 3)
VCOLS = 2 * L * NT         # 16384 virtual score cols per pair (tp-major)
UW = 1536                  # exp unit width (3 psum banks)
NU = (VCOLS + UW - 1) // UW  # 11 exp units per pair (10x1536 + 1x1024)

BF16 = mybir.dt.bfloat16
F32 = mybir.dt.float32
AF = mybir.ActivationFunctionType
ALU = mybir.AluOpType


def _split_excess_waits(nc, max_waits=1):
    """This toolchain's walrus accepts at most one sync wait per
    instruction; Tile's scheduler emits up to ~3. Move excess waits onto
    preceding same-engine nofuse NoOps (pure dispatch delay, semantics
    preserved)."""
    nsplit = 0
    for f in nc.m.functions:
        for blk in f.blocks:
            insts = list(blk.instructions)
            new = []
            changed = False
            for inst in insts:
                si = inst.sync_info
                if si is not None and si.on_wait and len(si.on_wait) > max_waits:
                    waits = list(si.on_wait)
                    excess = waits[:-max_waits]
                    for k in range(0, len(excess), max_waits):
                        nop = mybir.InstNoOp(
                            name=nc.get_next_instruction_name(), ins=[], outs=[]
                        )
                        nop.engine = inst.engine
                        nop.bass_nofuse = True
                        nop.sync_info = mybir.SyncInfo(
                            on_wait=excess[k : k + max_waits], on_update=[]
                        )
                        new.append(nop)
                        nsplit += 1
                    si.on_wait = waits[-max_waits:]
                    changed = True
                new.append(inst)
            if changed:
                blk.instructions = new
    return nsplit


def _build_program():
    nc = bass.Bass()
    qcat_d = nc.declare_dram_parameter("qcat", [PPC, 128, L], BF16, isOutput=False)
    kre_d = nc.declare_dram_parameter("kre", [PPC, 128, S], BF16, isOutput=False)
    kim_d = nc.declare_dram_parameter("kim", [PPC, 128, S], BF16, isOutput=False)
    vaug_d = nc.declare_dram_parameter("vaug", [PPC, 128, NT, VW], BF16, isOutput=False)
    vaug2_d = nc.declare_dram_parameter("vaug2", [PPC, 128, NT, VW], BF16, isOutput=False)
    out_d = nc.declare_dram_parameter("out", [PPC, NL, 128, 128], F32, isOutput=True)

    with TileContext(nc) as tc:
        with (
            tc.tile_pool(name="io", bufs=2) as io,
            tc.tile_pool(name="pp", bufs=2) as pp,
            tc.tile_pool(name="nrm", bufs=4) as nrm,
            tc.tile_pool(name="ob", bufs=4) as ob,
            tc.tile_pool(name="pss", bufs=2, space="PSUM") as pss,
            tc.tile_pool(name="psa", bufs=2, space="PSUM") as psa,
        ):

            def pcol(t, part, c):
                """Virtual column of P^T for s-tile t, re/im part, l-chunk c.
                h-major layout: col = h*8192 + (part*NT + t)*512 + l_local."""
                return (c // 4) * 8192 + (part * NT + t) * 512 + (c % 4) * 128

            def emit_av(state, c):
                """AV + normalize + store for l-chunk c of a finished pair."""
                p_pair, va_t, va2_t, pair = state
                acc = psa.tile([128, 258], F32, tag="acc")
                pr = acc[:, 0:129]
                pi = acc[:, 129:258]
                for t in range(NT):
                    o0 = pcol(t, 0, c)
                    nc.tensor.matmul(
                        pr,
                        lhsT=p_pair[:, o0 : o0 + 128],
                        rhs=va_t[:, t, 0:129],
                        start=(t == 0),
                        stop=(t == NT - 1),
                    )
                # pr half done: fold pr/Zr while the pi matmuls run
                rr = nrm.tile([128, 1], F32, tag="rr")
                nc.vector.reciprocal(rr, acc[:, 128:129])
                t1 = nrm.tile([128, 128], F32, tag="t1")
                nc.vector.tensor_scalar_mul(t1, acc[:, 0:128], rr)
                for t in range(NT):
                    o1 = pcol(t, 1, c)
                    nc.tensor.matmul(
                        pi,
                        lhsT=p_pair[:, o1 : o1 + 128],
                        rhs=va2_t[:, t, 0:129],
                        start=(t == 0),
                        stop=(t == NT - 1),
                    )
                # pr = [Pr@vr | Pr@vi | Zr], pi = [-Pi@vi | Pi@vr | Zi]
                ri = nrm.tile([128, 1], F32, tag="ri")
                nc.vector.reciprocal(ri, acc[:, 257:258])
                o = ob.tile([128, 128], F32)
                # [Vre | Vim] = pr[:,0:128]/Zr + pi[:,0:128]/Zi
                nc.vector.scalar_tensor_tensor(
                    o, in0=acc[:, 129:257], scalar=ri, in1=t1,
                    op0=ALU.mult, op1=ALU.add,
                )
                nc.sync.dma_start(out=out_d[pair, c], in_=o)

            # warm the ACT Exp table during the DMA fill so the first real
            # exp doesn't pay the table load
            warm = nrm.tile([128, 1], F32, tag="warm", bufs=1)
            warm2 = nrm.tile([128, 1], BF16, tag="warm2", bufs=1)
            nc.vector.memset(warm, 0.0)
            nc.scalar.activation(out=warm2, in_=warm, func=AF.Exp)

            prev = None
            for pair in range(PPC):
                q_t = io.tile([128, L], BF16, tag="q")
                kre_t = io.tile([128, S], BF16, tag="kre")
                kim_t = io.tile([128, S], BF16, tag="kim")
                va_t = io.tile([128, NT, VW], BF16, tag="va", bufs=3)
                va2_t = io.tile([128, NT, VW], BF16, tag="va2", bufs=3)
                # split the first pair's k/q DMAs so exp unit 0 (which only
                # needs kre tiles 0-2 and q[0:512]) can start ASAP; kim is
                # not touched until unit ~3.
                if pair == 0:
                    nc.sync.dma_start(out=kre_t[:, 0:384], in_=kre_d[pair, :, 0:384])
                    nc.sync.dma_start(out=q_t[:, 0:512], in_=qcat_d[pair, :, 0:512])
                    nc.sync.dma_start(out=kre_t[:, 384:S], in_=kre_d[pair, :, 384:S])
                    nc.sync.dma_start(out=kim_t, in_=kim_d[pair])
                    nc.sync.dma_start(out=q_t[:, 512:L], in_=qcat_d[pair, :, 512:L])
                else:
                    nc.sync.dma_start(out=kre_t, in_=kre_d[pair])
                    nc.sync.dma_start(out=q_t, in_=qcat_d[pair])
                    nc.sync.dma_start(out=kim_t, in_=kim_d[pair])
                nc.sync.dma_start(out=va_t, in_=vaug_d[pair])
                nc.sync.dma_start(out=va2_t, in_=vaug2_d[pair])

                # one [128, VCOLS] bf16 tile holds the pair's whole P^T
                # stream, h-major: col = h*8192 + (part*NT+t)*512 + l_local
                p_pair = pp.tile([128, VCOLS], BF16, tag="p")
                cur = (p_pair, va_t, va2_t, pair)

                plan = [(3 * i, 3) for i in range(10)] + [(30, 2)]
                nu = len(plan)
                for u in range(nu):
                    g0, nchunk = plan[u]
                    ps = pss.tile([128, UW], F32, tag="ps")
                    for j in range(nchunk):
                        g = g0 + j             # global 512-col chunk index
                        h, tp = g // 16, g % 16
                        part, t = tp // NT, tp % NT
                        ksrc = kre_t if part == 0 else kim_t
                        nc.tensor.matmul(
                            ps[:, j * 512 : (j + 1) * 512],
                            lhsT=ksrc[:, t * 128 : (t + 1) * 128],
                            rhs=q_t[:, h * 512 : (h + 1) * 512],
                            start=True,
                            stop=True,
                        )
                    nc.scalar.activation(
                        out=p_pair[:, g0 * 512 : (g0 + nchunk) * 512],
                        in_=ps[:, 0 : nchunk * 512],
                        func=AF.Exp,
                    )
                    # chunks 4-7 of the previous pair early in this slot;
                    # chunks 0-3 of this pair (l<512, exp'd once chunk 15
                    # is done after unit nu-6) late.
                    if prev is not None and 1 <= u <= 4:
                        emit_av(prev, u + 3)
                    if nu - 5 <= u <= nu - 2:
                        emit_av(cur, u - (nu - 5))
                prev = cur
            for c in range(4, NL):
                emit_av(prev, c)

    _split_excess_waits(nc)
    return nc


_CACHED_NC = None


def _get_program():
    global _CACHED_NC
    if _CACHED_NC is None:
        _CACHED_NC = _build_program()
    return _CACHED_NC


def _prep_in_maps(inputs):
    return _prep(
        inputs["q_real"], inputs["q_imag"], inputs["k_real"], inputs["k_imag"],
        inputs["v_real"], inputs["v_imag"],
    )


def _prep(q_real, q_imag, k_real, k_imag, v_real, v_imag):
    bf16 = ml_dtypes.bfloat16
    scale = 1.0 / np.sqrt(E)

    # [B,L,H,E] -> [B,H,E,L]; pack re/im along E into 128 partitions
    qr_t = np.transpose(np.asarray(q_real, np.float32), (0, 2, 3, 1))
    qi_t = np.transpose(np.asarray(q_imag, np.float32), (0, 2, 3, 1))
    qcat = (np.concatenate([qr_t, qi_t], axis=2) * scale).astype(bf16)  # [B,H,128,L]

    kr_t = np.transpose(np.asarray(k_real, np.float32), (0, 2, 3, 1))
    ki_t = np.transpose(np.asarray(k_imag, np.float32), (0, 2, 3, 1))
    kre = np.concatenate([kr_t, ki_t], axis=2).astype(bf16)             # [B,H,128,S]
    kim = np.concatenate([-ki_t, kr_t], axis=2).astype(bf16)

    vr_t = np.transpose(np.asarray(v_real, np.float32), (0, 2, 1, 3))   # [B,H,S,D]
    vi_t = np.transpose(np.asarray(v_imag, np.float32), (0, 2, 1, 3))
    vaug = np.zeros((B, H, S, VW), np.float32)
    vaug[..., 0:D] = vr_t
    vaug[..., D : 2 * D] = vi_t
    vaug[..., 2 * D] = 1.0
    vaug2 = np.zeros((B, H, S, VW), np.float32)
    vaug2[..., 0:D] = -vi_t
    vaug2[..., D : 2 * D] = vr_t
    vaug2[..., 2 * D] = 1.0
    # [B,H,S,VW] -> [B,H,NT,128,VW] -> partition-major [B,H,128,NT,VW]
    vaug = np.transpose(vaug.reshape(B, H, NT, 128, VW), (0, 1, 3, 2, 4)).astype(bf16)
    vaug2 = np.transpose(vaug2.reshape(B, H, NT, 128, VW), (0, 1, 3, 2, 4)).astype(bf16)

    qcat = qcat.reshape(PAIRS, 128, L)
    kre = kre.reshape(PAIRS, 128, S)
    kim = kim.reshape(PAIRS, 128, S)
    vaug = vaug.reshape(PAIRS, 128, NT, VW)
    vaug2 = vaug2.reshape(PAIRS, 128, NT, VW)

    in_maps = []
    for c in range(NCORES):
        sl = slice(c * PPC, (c + 1) * PPC)
        in_maps.append(
            {
                "qcat": np.ascontiguousarray(qcat[sl]),
                "kre": np.ascontiguousarray(kre[sl]),
                "kim": np.ascontiguousarray(kim[sl]),
                "vaug": np.ascontiguousarray(vaug[sl]),
                "vaug2": np.ascontiguousarray(vaug2[sl]),
            }
        )
    return in_maps


def kernel(q_real, q_imag, k_real, k_imag, v_real, v_imag, attn_mask=None):
    in_maps = _prep(q_real, q_imag, k_real, k_imag, v_real, v_imag)
    nc = _get_program()
    res = run_bass_kernel_spmd(nc, in_maps, list(range(NCORES)))
    outs = np.concatenate(
        [res.results[c]["out"] for c in range(NCORES)], axis=0
    )  # [32, NL, 128, 128]
    outs = outs.reshape(B, H, L, 128)
    v_re = np.transpose(outs[..., 0:D], (0, 2, 1, 3))   # [B,L,H,D]
    v_im = np.transpose(outs[..., D : 2 * D], (0, 2, 1, 3))
    return np.stack([v_re, v_im], axis=0).astype(np.float32)


# revision 13
# speedup vs baseline: 1.0057x; 1.0057x over previous
"""Complex attention (split re/im softmax) on 8 trn2 NeuronCores.

Math per (b,h) pair (L=S=1024, E=D=64):
  scores_re[l,s] = sum_e qr[l,e]kr[s,e] + qi[l,e]ki[s,e]   (x 1/sqrt(E))
  scores_im[l,s] = sum_e qi[l,e]kr[s,e] - qr[l,e]ki[s,e]   (x 1/sqrt(E))
  Ar = softmax_s(scores_re); Ai = softmax_s(scores_im)
  Vre = Ar@vr - Ai@vi ; Vim = Ar@vi + Ai@vr

Kernel strategy (per core: 4 of the 32 (b,h) pairs):
  - Pack the re/im contraction into K=128 matmuls:
      qcat = [qr; qi]^T * scale   [128, L]
      kre  = [kr; ki]^T           [128, S]
      kim  = [-ki; kr]^T          [128, S]
    scoresT (s on partitions, l free) = kre_chunk.T @ qcat, kim_chunk.T @ qcat
  - The P^T = exp(scoresT) stream is the bottleneck: exp only runs on the
    Scalar engine at 1 col/cycle.  Scores psum is organized as a rotating
    pair of [128,1536] tiles (6 banks) so the Tensor engine runs ~2 units
    ahead of ScalarE, and exp units are as wide as psum allows (fewer
    fixed per-instruction overheads).
  - AV for pair p-1 is interleaved into pair p's score stream:
      acc[l,0:129]   += Pre^T_chunk.T @ [vr  | vi | 1]
      acc[l,129:258] += Pim^T_chunk.T @ [-vi | vr | 1]
    (both halves live in one psum bank; col 128/257 are the softmax
    denominators Zr/Zi).  The sign flip in vaug2 makes the final combine a
    single fused op per half:
      V = acc_re[:,0:128]/Zr + acc_im[:,0:128]/Zi = [Vre | Vim]
  - Normalize on VectorE: 2 reciprocals + tensor_scalar mul + one
    scalar_tensor_tensor, then DMA [128,128] f32 per l-chunk.
"""

import numpy as np
import ml_dtypes

import concourse.bass as bass
from concourse import mybir
from concourse.tile import TileContext
from concourse.bass_utils import run_bass_kernel_spmd

B, L, H, E = 4, 1024, 8, 64
S, D = 1024, 64
NCORES = 8
PAIRS = B * H              # 32 (b,h) pairs
PPC = PAIRS // NCORES      # 4 pairs per core
NT = S // 128              # 8 s-tiles
NL = L // 128              # 8 l-chunks
NTP = 2 * NT               # 16 tile-parts (s-tile x re/im) per pair
VW = 132                   # padded vaug width (v0 64 | v1 64 | ones 1 | pad 3)
VCOLS = 2 * L * NT         # 16384 virtual score cols per pair (tp-major)
UW = 1536                  # exp unit width (3 psum banks)
NU = (VCOLS + UW - 1) // UW  # 11 exp units per pair (10x1536 + 1x1024)

BF16 = mybir.dt.bfloat16
F32 = mybir.dt.float32
AF = mybir.ActivationFunctionType
ALU = mybir.AluOpType


def _split_excess_waits(nc, max_waits=1):
    """This toolchain's walrus accepts at most one sync wait per
    instruction; Tile's scheduler emits up to ~3. Move excess waits onto
    preceding same-engine nofuse NoOps (pure dispatch delay, semantics
    preserved)."""
    nsplit = 0
    for f in nc.m.functions:
        for blk in f.blocks:
            insts = list(blk.instructions)
            new = []
            changed = False
            for inst in insts:
                si = inst.sync_info
                if si is not None and si.on_wait and len(si.on_wait) > max_waits:
                    waits = list(si.on_wait)
                    excess = waits[:-max_waits]
                    for k in range(0, len(excess), max_waits):
                        nop = mybir.InstNoOp(
                            name=nc.get_next_instruction_name(), ins=[], outs=[]
                        )
                        nop.engine = inst.engine
                        nop.bass_nofuse = True
                        nop.sync_info = mybir.SyncInfo(
                            on_wait=excess[k : k + max_waits], on_update=[]
                        )
                        new.append(nop)
                        nsplit += 1
                    si.on_wait = waits[-max_waits:]
                    changed = True
                new.append(inst)
            if changed:
                blk.instructions = new
    return nsplit


def _build_program():
    nc = bass.Bass()
    qcat_d = nc.declare_dram_parameter("qcat", [PPC, 128, L], BF16, isOutput=False)
    kre_d = nc.declare_dram_parameter("kre", [PPC, 128, S], BF16, isOutput=False)
    kim_d = nc.declare_dram_parameter("kim", [PPC, 128, S], BF16, isOutput=False)
    vaug_d = nc.declare_dram_parameter("vaug", [PPC, 128, NT, VW], BF16, isOutput=False)
    vaug2_d = nc.declare_dram_parameter("vaug2", [PPC, 128, NT, VW], BF16, isOutput=False)
    out_d = nc.declare_dram_parameter("out", [PPC, NL, 128, 128], F32, isOutput=True)

    with TileContext(nc) as tc:
        with (
            tc.tile_pool(name="io", bufs=2) as io,
            tc.tile_pool(name="pp", bufs=2) as pp,
            tc.tile_pool(name="nrm", bufs=4) as nrm,
            tc.tile_pool(name="ob", bufs=4) as ob,
            tc.tile_pool(name="pss", bufs=2, space="PSUM") as pss,
            tc.tile_pool(name="psa", bufs=2, space="PSUM") as psa,
        ):

            def pcol(t, part, c):
                """Virtual column of P^T for s-tile t, re/im part, l-chunk c.
                h-major layout: col = h*8192 + (part*NT + t)*512 + l_local."""
                return (c // 4) * 8192 + (part * NT + t) * 512 + (c % 4) * 128

            def emit_av(state, c):
                """AV + normalize + store for l-chunk c of a finished pair."""
                p_pair, va_t, va2_t, pair = state
                acc = psa.tile([128, 258], F32, tag="acc")
                pr = acc[:, 0:129]
                pi = acc[:, 129:258]
                for t in range(NT):
                    o0 = pcol(t, 0, c)
                    nc.tensor.matmul(
                        pr,
                        lhsT=p_pair[:, o0 : o0 + 128],
                        rhs=va_t[:, t, 0:129],
                        start=(t == 0),
                        stop=(t == NT - 1),
                    )
                # pr half done: fold pr/Zr while the pi matmuls run
                rr = nrm.tile([128, 1], F32, tag="rr")
                nc.vector.reciprocal(rr, acc[:, 128:129])
                t1 = nrm.tile([128, 128], F32, tag="t1")
                nc.vector.tensor_scalar_mul(t1, acc[:, 0:128], rr)
                for t in range(NT):
                    o1 = pcol(t, 1, c)
                    nc.tensor.matmul(
                        pi,
                        lhsT=p_pair[:, o1 : o1 + 128],
                        rhs=va2_t[:, t, 0:129],
                        start=(t == 0),
                        stop=(t == NT - 1),
                    )
                # pr = [Pr@vr | Pr@vi | Zr], pi = [-Pi@vi | Pi@vr | Zi]
                ri = nrm.tile([128, 1], F32, tag="ri")
                nc.vector.reciprocal(ri, acc[:, 257:258])
                o = ob.tile([128, 128], F32)
                # [Vre | Vim] = pr[:,0:128]/Zr + pi[:,0:128]/Zi
                nc.vector.scalar_tensor_tensor(
                    o, in0=acc[:, 129:257], scalar=ri, in1=t1,
                    op0=ALU.mult, op1=ALU.add,
                )
                nc.sync.dma_start(out=out_d[pair, c], in_=o)

            # warm the ACT Exp table during the DMA fill so the first real
            # exp doesn't pay the table load
            warm = nrm.tile([128, 1], F32, tag="warm", bufs=1)
            warm2 = nrm.tile([128, 1], BF16, tag="warm2", bufs=1)
            nc.vector.memset(warm, 0.0)
            nc.scalar.activation(out=warm2, in_=warm, func=AF.Exp)

            prev = None
            for pair in range(PPC):
                q_t = io.tile([128, L], BF16, tag="q")
                kre_t = io.tile([128, S], BF16, tag="kre")
                kim_t = io.tile([128, S], BF16, tag="kim")
                va_t = io.tile([128, NT, VW], BF16, tag="va", bufs=3)
                va2_t = io.tile([128, NT, VW], BF16, tag="va2", bufs=3)
                # split the first pair's k/q DMAs so exp unit 0 (which only
                # needs kre tiles 0-2 and q[0:512]) can start ASAP; kim is
                # not touched until unit ~3.
                if pair == 0:
                    nc.sync.dma_start(out=kre_t[:, 0:384], in_=kre_d[pair, :, 0:384])
                    nc.sync.dma_start(out=q_t[:, 0:512], in_=qcat_d[pair, :, 0:512])
                    nc.sync.dma_start(out=kre_t[:, 384:S], in_=kre_d[pair, :, 384:S])
                    nc.sync.dma_start(out=kim_t, in_=kim_d[pair])
                    nc.sync.dma_start(out=q_t[:, 512:L], in_=qcat_d[pair, :, 512:L])
                else:
                    nc.sync.dma_start(out=kre_t, in_=kre_d[pair])
                    nc.sync.dma_start(out=q_t, in_=qcat_d[pair])
                    nc.sync.dma_start(out=kim_t, in_=kim_d[pair])
                nc.sync.dma_start(out=va_t, in_=vaug_d[pair])
                nc.sync.dma_start(out=va2_t, in_=vaug2_d[pair])

                # one [128, VCOLS] bf16 tile holds the pair's whole P^T
                # stream, h-major: col = h*8192 + (part*NT+t)*512 + l_local
                p_pair = pp.tile([128, VCOLS], BF16, tag="p")
                cur = (p_pair, va_t, va2_t, pair)

                if pair == 0:
                    plan = [(0, 1), (1, 2)] + [(3 + 3 * i, 3) for i in range(9)] + [(30, 2)]
                else:
                    plan = [(3 * i, 3) for i in range(10)] + [(30, 2)]
                nu = len(plan)
                for u in range(nu):
                    g0, nchunk = plan[u]
                    ps = pss.tile([128, UW], F32, tag="ps")
                    for j in range(nchunk):
                        g = g0 + j             # global 512-col chunk index
                        h, tp = g // 16, g % 16
                        part, t = tp // NT, tp % NT
                        ksrc = kre_t if part == 0 else kim_t
                        nc.tensor.matmul(
                            ps[:, j * 512 : (j + 1) * 512],
                            lhsT=ksrc[:, t * 128 : (t + 1) * 128],
                            rhs=q_t[:, h * 512 : (h + 1) * 512],
                            start=True,
                            stop=True,
                        )
                    nc.scalar.activation(
                        out=p_pair[:, g0 * 512 : (g0 + nchunk) * 512],
                        in_=ps[:, 0 : nchunk * 512],
                        func=AF.Exp,
                    )
                    # chunks 4-7 of the previous pair early in this slot;
                    # chunks 0-3 of this pair (l<512, exp'd once chunk 15
                    # is done after unit nu-6) late.
                    if prev is not None and 1 <= u <= 4:
                        emit_av(prev, u + 3)
                    if nu - 5 <= u <= nu - 2:
                        emit_av(cur, u - (nu - 5))
                prev = cur
            for c in range(4, NL):
                emit_av(prev, c)

    _split_excess_waits(nc)
    return nc


_CACHED_NC = None


def _get_program():
    global _CACHED_NC
    if _CACHED_NC is None:
        _CACHED_NC = _build_program()
    return _CACHED_NC


def _prep_in_maps(inputs):
    return _prep(
        inputs["q_real"], inputs["q_imag"], inputs["k_real"], inputs["k_imag"],
        inputs["v_real"], inputs["v_imag"],
    )


def _prep(q_real, q_imag, k_real, k_imag, v_real, v_imag):
    bf16 = ml_dtypes.bfloat16
    scale = 1.0 / np.sqrt(E)

    # [B,L,H,E] -> [B,H,E,L]; pack re/im along E into 128 partitions
    qr_t = np.transpose(np.asarray(q_real, np.float32), (0, 2, 3, 1))
    qi_t = np.transpose(np.asarray(q_imag, np.float32), (0, 2, 3, 1))
    qcat = (np.concatenate([qr_t, qi_t], axis=2) * scale).astype(bf16)  # [B,H,128,L]

    kr_t = np.transpose(np.asarray(k_real, np.float32), (0, 2, 3, 1))
    ki_t = np.transpose(np.asarray(k_imag, np.float32), (0, 2, 3, 1))
    kre = np.concatenate([kr_t, ki_t], axis=2).astype(bf16)             # [B,H,128,S]
    kim = np.concatenate([-ki_t, kr_t], axis=2).astype(bf16)

    vr_t = np.transpose(np.asarray(v_real, np.float32), (0, 2, 1, 3))   # [B,H,S,D]
    vi_t = np.transpose(np.asarray(v_imag, np.float32), (0, 2, 1, 3))
    vaug = np.zeros((B, H, S, VW), np.float32)
    vaug[..., 0:D] = vr_t
    vaug[..., D : 2 * D] = vi_t
    vaug[..., 2 * D] = 1.0
    vaug2 = np.zeros((B, H, S, VW), np.float32)
    vaug2[..., 0:D] = -vi_t
    vaug2[..., D : 2 * D] = vr_t
    vaug2[..., 2 * D] = 1.0
    # [B,H,S,VW] -> [B,H,NT,128,VW] -> partition-major [B,H,128,NT,VW]
    vaug = np.transpose(vaug.reshape(B, H, NT, 128, VW), (0, 1, 3, 2, 4)).astype(bf16)
    vaug2 = np.transpose(vaug2.reshape(B, H, NT, 128, VW), (0, 1, 3, 2, 4)).astype(bf16)

    qcat = qcat.reshape(PAIRS, 128, L)
    kre = kre.reshape(PAIRS, 128, S)
    kim = kim.reshape(PAIRS, 128, S)
    vaug = vaug.reshape(PAIRS, 128, NT, VW)
    vaug2 = vaug2.reshape(PAIRS, 128, NT, VW)

    in_maps = []
    for c in range(NCORES):
        sl = slice(c * PPC, (c + 1) * PPC)
        in_maps.append(
            {
                "qcat": np.ascontiguousarray(qcat[sl]),
                "kre": np.ascontiguousarray(kre[sl]),
                "kim": np.ascontiguousarray(kim[sl]),
                "vaug": np.ascontiguousarray(vaug[sl]),
                "vaug2": np.ascontiguousarray(vaug2[sl]),
            }
        )
    return in_maps


def kernel(q_real, q_imag, k_real, k_imag, v_real, v_imag, attn_mask=None):
    in_maps = _prep(q_real, q_imag, k_real, k_imag, v_real, v_imag)
    nc = _get_program()
    res = run_bass_kernel_spmd(nc, in_maps, list(range(NCORES)))
    outs = np.concatenate(
        [res.results[c]["out"] for c in range(NCORES)], axis=0
    )  # [32, NL, 128, 128]
    outs = outs.reshape(B, H, L, 128)
    v_re = np.transpose(outs[..., 0:D], (0, 2, 1, 3))   # [B,L,H,D]
    v_im = np.transpose(outs[..., D : 2 * D], (0, 2, 1, 3))
    return np.stack([v_re, v_im], axis=0).astype(np.float32)
